# revision 16
# baseline (speedup 1.0000x reference)
# Multi-head attention kernel for Trainium2, sharded over 8 NeuronCores.
#
# Sharding: core = (batch b, query-chunk qc). Each core handles QB=512 queries
# of one batch, all 12 heads, recomputing the K/V projections for its batch
# (cheaper than cross-core collectives on this chip).
#
# Layout strategy (bf16 matmul operands, fp32 PSUM accumulation/epilogues):
#   - Host pre-transposes activations to [E, S] so the contraction dim (E)
#     lands on SBUF partitions; fp32 matmul is avoided on-device (it lowers to
#     two PE passes), so all matmul operands are bf16.
#   - q^T, k^T computed as [768, S] via lhsT=W chunks; per-partition bias
#     added during the PSUM->SBUF copy (DVE tensor_scalar, bf16 output).
#   - v computed directly as [keys, 768] using x_v^T chunks as the stationary
#     operand; stored with a ones-column per head ([128,16,12,65]) so the PV
#     matmul (M=65) also produces the softmax denominator row for free.
#   - scores^T = [keys, queries] per head: K=64 matmuls; even/odd heads sit in
#     partition halves 0-63/64-127, emitted adjacently so they land in
#     disjoint PE row groups and run concurrently (row packing).
#   - exp on ScalarE in [128, 2x512] groups PSUM->SBUF (bf16), streamed
#     straight into the accumulating PV matmul (no full score matrix in SBUF).
#   - softmax normalize: per-pair staging copies + SBUF->SBUF gather of the 12
#     denominator rows, one batched DVE reciprocal, per-head PE broadcast
#     (selector matmul) and DVE multiply. Note: accumulating matmul groups
#     must not mix tile positions (HW), hence K=128 head-pair contractions in
#     the output projection; DVE ops need 32-aligned base partitions.
#   - output projection contracts head pairs as K=128 matmuls; epilogue adds
#     host-precomputed bias (bv folded through Wo + bo).

import numpy as np
from contextlib import ExitStack

import concourse.bass as bass
import concourse.mybir as mybir
import concourse.tile as tile
from concourse import bacc
from concourse.bass_utils import run_bass_kernel_spmd

F32 = mybir.dt.float32
BF16 = mybir.dt.bfloat16
F16 = mybir.dt.float16
P = 128
E = 768
S = 2048
B = 2
H = 12
D = 64
QB = 512          # queries per core
NCORES = 8
EC = E // P       # 6 e-chunks
KT = S // P       # 16 key tiles
MT_Q = E // P     # 6 M-tiles for q^T/k^T (768 rows)
NC4 = S // 512    # 4 n-slices of k^T


def build_nc():
    nc = bacc.Bacc("TRN2", debug=False)

    # DRAM I/O (per-core shapes; same NEFF on all 8 cores)
    xq = nc.dram_tensor("xq", (E, QB), BF16, kind="ExternalInput")     # query[b,chunk].T
    xk = nc.dram_tensor("xk", (E, S), BF16, kind="ExternalInput")      # key[b].T
    xv = nc.dram_tensor("xv", (E, S), BF16, kind="ExternalInput")      # value[b].T
    wq = nc.dram_tensor("wq", (E, E), BF16, kind="ExternalInput")      # [E, H*D], pre-scaled 1/sqrt(D)
    wk = nc.dram_tensor("wk", (E, E), BF16, kind="ExternalInput")
    wv = nc.dram_tensor("wv", (E, E), BF16, kind="ExternalInput")
    wo = nc.dram_tensor("wo", (E, E), BF16, kind="ExternalInput")
    bq = nc.dram_tensor("bq", (P, MT_Q), F32, kind="ExternalInput")   # per-partition bias per M-tile
    bk = nc.dram_tensor("bk", (P, MT_Q), F32, kind="ExternalInput")
    bo = nc.dram_tensor("bo", (P, E), F32, kind="ExternalInput")      # bv@Wo + bo, broadcast
    seld = nc.dram_tensor("seld", (2, P), F16, kind="ExternalInput")  # pair-broadcast selector
    out = nc.dram_tensor("out", (QB, E), F32, kind="ExternalOutput")

    with tile.TileContext(nc) as tc:
        with ExitStack() as ctx:
            _emit(ctx, tc, nc, xq, xk, xv, wq, wk, wv, wo, bq, bk, bo, seld, out)
    nc.compile()
    return nc


def _emit(ctx, tc, nc, xq, xk, xv, wq, wk, wv, wo, bq, bk, bo, seld, out):
    # ---- pools ----
    # SBUF persistent
    persist = ctx.enter_context(tc.tile_pool(name="persist", bufs=1))
    # big weight slots [128, 6, 768] reused wq -> wk -> wv -> wo
    wpool = ctx.enter_context(tc.tile_pool(name="wpool", bufs=2))
    # x input slices
    xpool = ctx.enter_context(tc.tile_pool(name="xpool", bufs=2))
    xvpool = ctx.enter_context(tc.tile_pool(name="xvpool", bufs=3))
    # exp output stream
    epool = ctx.enter_context(tc.tile_pool(name="epool", bufs=4))
    # small temps
    spool = ctx.enter_context(tc.tile_pool(name="spool", bufs=2))
    outpool = ctx.enter_context(tc.tile_pool(name="outpool", bufs=2))
    # PSUM pools
    psA = ctx.enter_context(tc.tile_pool(name="psA", bufs=2, space="PSUM"))   # [128,512] proj qk + PV out
    psB = ctx.enter_context(tc.tile_pool(name="psB", bufs=1, space="PSUM"))   # [128,768] v proj + out proj
    psC = ctx.enter_context(tc.tile_pool(name="psC", bufs=2, space="PSUM"))   # [128,2,512] scores

    # ---- persistent SBUF tensors ----
    qT = persist.tile([P, MT_Q, QB], BF16)       # q^T [768, QB]
    kT = persist.tile([P, MT_Q, S], BF16)        # k^T [768, S]
    v_sb = persist.tile([P, KT, H, D + 1], BF16)  # v + ones column per head
    o_all = persist.tile([P, H // 2, QB], BF16)   # normalized o^T, head pairs in partition halves
    bq_sb = persist.tile([P, MT_Q], F32)
    bk_sb = persist.tile([P, MT_Q], F32)
    bo_sb = persist.tile([P, E], F32)
    o_raw = persist.tile([D + 1, H, 512], F32)   # unnormalized o^T + denom row per head
    dens = persist.tile([2, H // 2, 512], F32)   # denominators, pair-packed [head-in-pair, hp]
    drec = persist.tile([2, H // 2, 512], F16)   # their reciprocals (fp16 matmul operand)
    sel2 = persist.tile([2, P], F16)             # pair broadcast: [0,0:64]=1, [1,64:128]=1

    # first-needed DMAs first; constants go on the scalar HWDGE queue
    wq_t = wpool.tile([P, EC, E], BF16, tag="w18")
    xq_t = xpool.tile([P, EC, QB], BF16, tag="xs")
    for ec in range(EC):
        nc.sync.dma_start(wq_t[:, ec, :], wq[ec * P:(ec + 1) * P, :])
        nc.sync.dma_start(xq_t[:, ec, :], xq[ec * P:(ec + 1) * P, :])
    nc.scalar.dma_start(bq_sb[:], bq[:])
    nc.scalar.dma_start(bk_sb[:], bk[:])
    nc.scalar.dma_start(bo_sb[:], bo[:])

    # ones columns for denominator (written once; v-proj copies don't touch col D)
    nc.vector.memset(v_sb[:, :, :, D], 1.0)
    # pair-broadcast selector: one K=2 fp16 matmul turns [2,512] reciprocals into
    # a [128,512] per-pair broadcast (head0 rows 0-63, head1 rows 64-127)
    nc.scalar.dma_start(sel2[:], seld[:])

    # ---- q^T projection ----
    for mt in range(MT_Q):
        ps = psA.tile([P, 512], F32, tag="psA")
        for ec in range(EC):
            nc.tensor.matmul(ps[:], wq_t[:, ec, mt * P:(mt + 1) * P], xq_t[:, ec, :],
                             start=(ec == 0), stop=(ec == EC - 1))
        nc.vector.tensor_scalar_add(qT[:, mt, :], ps[:], bq_sb[:, mt:mt + 1])

    # ---- k^T projection ----
    wk_t = wpool.tile([P, EC, E], BF16, tag="w18")
    for ec in range(EC):
        nc.sync.dma_start(wk_t[:, ec, :], wk[ec * P:(ec + 1) * P, :])
    for n4 in range(NC4):
        xk_t = xpool.tile([P, EC, 512], BF16, tag="xs")
        nc.sync.dma_start(xk_t[:], xk[:, n4 * 512:(n4 + 1) * 512].rearrange("(ec p) s -> p ec s", p=P))
        for mt in range(MT_Q):
            ps = psA.tile([P, 512], F32, tag="psA")
            for ec in range(EC):
                nc.tensor.matmul(ps[:], wk_t[:, ec, mt * P:(mt + 1) * P], xk_t[:, ec, :],
                                 start=(ec == 0), stop=(ec == EC - 1))
            nc.vector.tensor_scalar_add(kT[:, mt, n4 * 512:(n4 + 1) * 512], ps[:], bk_sb[:, mt:mt + 1])

    # ---- v projection (direct [keys, d]; no bias — folded into bo host-side) ----
    wv_t = wpool.tile([P, EC, E], BF16, tag="w18")
    for ec in range(EC):
        nc.sync.dma_start(wv_t[:, ec, :], wv[ec * P:(ec + 1) * P, :])
    for kt in range(KT):
        xv_t = xvpool.tile([P, EC, P], BF16, tag="xv")
        nc.sync.dma_start(xv_t[:], xv[:, kt * P:(kt + 1) * P].rearrange("(ec p) s -> p ec s", p=P))
        psv = psB.tile([P, E], F32, tag="psB")
        for ec in range(EC):
            nc.tensor.matmul(psv[:, 0:512], xv_t[:, ec, :], wv_t[:, ec, 0:512],
                             start=(ec == 0), stop=(ec == EC - 1))
            nc.tensor.matmul(psv[:, 512:768], xv_t[:, ec, :], wv_t[:, ec, 512:768],
                             start=(ec == 0), stop=(ec == EC - 1))
        # strided copy into per-head slots (leaves ones column intact)
        nc.vector.tensor_copy(v_sb[:, kt, :, 0:D], psv[:].rearrange("p (h d) -> p h d", d=D))

    # ---- attention: head pairs ----
    # Per key tile: both heads' score matmuls are adjacent K=64 ops on
    # disjoint PE row groups (partitions 0-63 / 64-127) -> run concurrently.
    for hp in range(H // 2):
        o_ps = {}
        for i in range(2):
            o_ps[i] = psA.tile([P, 512], F32, tag="psA", name=f"o_ps{i}")
        for kt in range(KT):
            st = psC.tile([P, 2, 512], F32, tag="psC")
            for i in range(2):
                po = D * i      # partition offset of this head's d-rows
                nc.tensor.matmul(st[:, i, :],
                                 kT[po:po + D, hp, kt * P:(kt + 1) * P],
                                 qT[po:po + D, hp, :],
                                 start=True, stop=True)
            ex = epool.tile([P, 2, 512], BF16, tag="ex")
            nc.scalar.activation(ex[:, :, :], st[:, :, :], mybir.ActivationFunctionType.Exp)
            for i in range(2):
                nc.tensor.matmul(o_ps[i][0:D + 1, :],
                                 v_sb[:, kt, 2 * hp + i, :],
                                 ex[:, i, :],
                                 start=(kt == 0), stop=(kt == KT - 1))
        # stage unnormalized outputs (fast PSUM release) and gather denom rows
        for i in range(2):
            nc.vector.tensor_copy(o_raw[:, 2 * hp + i, :], o_ps[i][0:D + 1, :])
            nc.sync.dma_start(dens[i:i + 1, hp, :], o_raw[D:D + 1, 2 * hp + i, :])
        # in-stream softmax normalization for this head pair: one fp16 K=2
        # matmul broadcasts both reciprocals into partition halves at once
        with nc.allow_low_precision(reason="fp16 reciprocal feeds fp16 broadcast matmul; den ~1e3, ample range"):
            nc.vector.reciprocal(drec[:, hp, :], dens[:, hp, :])
        bc_ps = psA.tile([P, 512], F32, tag="psA", name="bc")
        nc.tensor.matmul(bc_ps[:], sel2[:], drec[:, hp, :], start=True, stop=True)
        for i in range(2):
            po = D * i
            nc.vector.tensor_tensor(o_all[po:po + D, hp, :], o_raw[0:D, 2 * hp + i, :],
                                    bc_ps[po:po + D, :], mybir.AluOpType.mult)

    # ---- output projection ----
    wo_t = wpool.tile([P, EC, E], BF16, tag="w18")
    nc.sync.dma_start(wo_t[:], wo[:].rearrange("(ec p) m -> p ec m", p=P))
    ST = QB // P  # 4 s-tiles
    for st4 in range(ST):
        op = psB.tile([P, E], F32, tag="psB")
        for hp in range(H // 2):
            # both heads of the pair contract in one K=128 matmul
            first = (hp == 0)
            last = (hp == H // 2 - 1)
            nc.tensor.matmul(op[:, 0:512],
                             o_all[:, hp, st4 * P:(st4 + 1) * P],
                             wo_t[:, hp, 0:512],
                             start=first, stop=last)
            nc.tensor.matmul(op[:, 512:768],
                             o_all[:, hp, st4 * P:(st4 + 1) * P],
                             wo_t[:, hp, 512:768],
                             start=first, stop=last)
        out_sb = outpool.tile([P, E], F32, tag="outsb")
        nc.vector.tensor_tensor(out_sb[:], op[:], bo_sb[:], mybir.AluOpType.add)
        nc.sync.dma_start(out[st4 * P:(st4 + 1) * P, :], out_sb[:])


_NC_CACHE = None


def _get_nc():
    global _NC_CACHE
    if _NC_CACHE is None:
        _NC_CACHE = build_nc()
    return _NC_CACHE


def make_in_maps(query, key_, value, Wq, bq, Wk, bk, Wv, bv, Wo, bo):
    """Host-side sharding + layout prep. Returns list of 8 input dicts."""
    query = np.asarray(query, dtype=np.float32)
    key_ = np.asarray(key_, dtype=np.float32)
    value = np.asarray(value, dtype=np.float32)
    scale = 1.0 / np.sqrt(np.float32(D))

    import ml_dtypes
    BF = ml_dtypes.bfloat16
    wq_f = (np.ascontiguousarray(np.transpose(np.asarray(Wq, np.float32), (1, 0, 2)).reshape(E, E)) * scale).astype(BF)
    wk_f = np.ascontiguousarray(np.transpose(np.asarray(Wk, np.float32), (1, 0, 2)).reshape(E, E)).astype(BF)
    wv_f = np.ascontiguousarray(np.transpose(np.asarray(Wv, np.float32), (1, 0, 2)).reshape(E, E)).astype(BF)
    wo_f = np.ascontiguousarray(np.asarray(Wo, np.float32)).astype(BF)

    bq_f = (np.asarray(bq, np.float32).reshape(E) * scale).reshape(MT_Q, P).T.copy()
    bk_f = np.asarray(bk, np.float32).reshape(E).reshape(MT_Q, P).T.copy()
    bv_f = np.asarray(bv, np.float32).reshape(E)
    wo_f32 = wo_f.astype(np.float32)
    bo_eff = np.tile((bv_f @ wo_f32 + np.asarray(bo, np.float32)).reshape(1, E), (P, 1)).copy()

    xk_t = [np.ascontiguousarray(key_[b].T).astype(BF) for b in range(B)]
    xv_t = [np.ascontiguousarray(value[b].T).astype(BF) for b in range(B)]

    sel_np = np.zeros((2, P), dtype=np.float16)
    sel_np[0, 0:D] = 1.0
    sel_np[1, D:2 * D] = 1.0

    in_maps = []
    for core in range(NCORES):
        b = core // (NCORES // B)
        qc = core % (NCORES // B)
        xq_t = np.ascontiguousarray(query[b, qc * QB:(qc + 1) * QB, :].T).astype(BF)
        in_maps.append({
            "xq": xq_t, "xk": xk_t[b], "xv": xv_t[b],
            "wq": wq_f, "wk": wk_f, "wv": wv_f, "wo": wo_f,
            "bq": bq_f, "bk": bk_f, "bo": bo_eff, "seld": sel_np,
        })
    return in_maps


def assemble(results):
    outp = np.empty((B, S, E), dtype=np.float32)
    for core in range(NCORES):
        b = core // (NCORES // B)
        qc = core % (NCORES // B)
        outp[b, qc * QB:(qc + 1) * QB, :] = results[core]["out"]
    return outp


def kernel(query, key_, value, Wq, bq, Wk, bk, Wv, bv, Wo, bo):
    nc = _get_nc()
    in_maps = make_in_maps(query, key_, value, Wq, bq, Wk, bk, Wv, bv, Wo, bo)
    res = run_bass_kernel_spmd(nc, in_maps, core_ids=list(range(NCORES)))
    return assemble(res.results)



# revision 17
# speedup vs baseline: 1.1318x; 1.1318x over previous
# Multi-head attention kernel for Trainium2, sharded over 8 NeuronCores.
#
# Sharding: core = (batch b, query-chunk qc). Each core handles QB=512 queries
# of one batch, all 12 heads, recomputing the K/V projections for its batch
# (cheaper than cross-core collectives on this chip).
#
# Layout strategy (bf16 matmul operands, fp32 PSUM accumulation/epilogues):
#   - Host pre-transposes activations to [E, S] so the contraction dim (E)
#     lands on SBUF partitions; fp32 matmul is avoided on-device (it lowers to
#     two PE passes), so all matmul operands are bf16.
#   - q^T, k^T computed as [768, S] via lhsT=W chunks; per-partition bias
#     added during the PSUM->SBUF copy (DVE tensor_scalar, bf16 output).
#   - v computed directly as [keys, 768] using x_v^T chunks as the stationary
#     operand; stored with a ones-column per head ([128,16,12,65]) so the PV
#     matmul (M=65) also produces the softmax denominator row for free.
#   - scores^T = [keys, queries] per head: K=64 matmuls; even/odd heads sit in
#     partition halves 0-63/64-127, emitted adjacently so they land in
#     disjoint PE row groups and run concurrently (row packing).
#   - exp on ScalarE in [128, 2x512] groups PSUM->SBUF (bf16), streamed
#     straight into the accumulating PV matmul (no full score matrix in SBUF).
#   - softmax normalize: per-pair staging copies + SBUF->SBUF gather of the 12
#     denominator rows, one batched DVE reciprocal, per-head PE broadcast
#     (selector matmul) and DVE multiply. Note: accumulating matmul groups
#     must not mix tile positions (HW), hence K=128 head-pair contractions in
#     the output projection; DVE ops need 32-aligned base partitions.
#   - output projection contracts head pairs as K=128 matmuls; epilogue adds
#     host-precomputed bias (bv folded through Wo + bo).

import numpy as np
from contextlib import ExitStack

import concourse.bass as bass
import concourse.mybir as mybir
import concourse.tile as tile
from concourse import bacc
from concourse.bass_utils import run_bass_kernel_spmd

F32 = mybir.dt.float32
BF16 = mybir.dt.bfloat16
F16 = mybir.dt.float16
P = 128
E = 768
S = 2048
B = 2
H = 12
D = 64
QB = 512          # queries per core
NCORES = 8
EC = E // P       # 6 e-chunks
KT = S // P       # 16 key tiles
MT_Q = E // P     # 6 M-tiles for q^T/k^T (768 rows)
NC4 = S // 512    # 4 n-slices of k^T


def build_nc():
    nc = bacc.Bacc("TRN2", debug=False)

    # DRAM I/O (per-core shapes; same NEFF on all 8 cores)
    xq = nc.dram_tensor("xq", (E, QB), BF16, kind="ExternalInput")     # query[b,chunk].T
    xk = nc.dram_tensor("xk", (E, S), BF16, kind="ExternalInput")      # key[b].T
    xv = nc.dram_tensor("xv", (E, S), BF16, kind="ExternalInput")      # value[b].T
    wq = nc.dram_tensor("wq", (E, E), BF16, kind="ExternalInput")      # [E, H*D], pre-scaled 1/sqrt(D)
    wk = nc.dram_tensor("wk", (E, E), BF16, kind="ExternalInput")
    wv = nc.dram_tensor("wv", (E, E), BF16, kind="ExternalInput")
    wo = nc.dram_tensor("wo", (E, E), BF16, kind="ExternalInput")
    bq = nc.dram_tensor("bq", (P, MT_Q), F32, kind="ExternalInput")   # per-partition bias per M-tile
    bk = nc.dram_tensor("bk", (P, MT_Q), F32, kind="ExternalInput")
    bo = nc.dram_tensor("bo", (P, E), F32, kind="ExternalInput")      # bv@Wo + bo, broadcast
    seld = nc.dram_tensor("seld", (2, P), F16, kind="ExternalInput")  # pair-broadcast selector
    out = nc.dram_tensor("out", (QB, E), F32, kind="ExternalOutput")

    with tile.TileContext(nc) as tc:
        with ExitStack() as ctx:
            _emit(ctx, tc, nc, xq, xk, xv, wq, wk, wv, wo, bq, bk, bo, seld, out)
    nc.compile()
    return nc


def _emit(ctx, tc, nc, xq, xk, xv, wq, wk, wv, wo, bq, bk, bo, seld, out):
    # ---- pools ----
    # SBUF persistent
    persist = ctx.enter_context(tc.tile_pool(name="persist", bufs=1))
    # big weight slots [128, 6, 768] reused wq -> wk -> wv -> wo
    wpool = ctx.enter_context(tc.tile_pool(name="wpool", bufs=2))
    # x input slices
    xpool = ctx.enter_context(tc.tile_pool(name="xpool", bufs=2))
    xvpool = ctx.enter_context(tc.tile_pool(name="xvpool", bufs=3))
    # exp output stream
    epool = ctx.enter_context(tc.tile_pool(name="epool", bufs=4))
    # small temps
    spool = ctx.enter_context(tc.tile_pool(name="spool", bufs=2))
    outpool = ctx.enter_context(tc.tile_pool(name="outpool", bufs=2))
    # PSUM pools
    psA = ctx.enter_context(tc.tile_pool(name="psA", bufs=2, space="PSUM"))   # [128,512] proj qk + PV out
    psB = ctx.enter_context(tc.tile_pool(name="psB", bufs=1, space="PSUM"))   # [128,768] v proj + out proj
    psC = ctx.enter_context(tc.tile_pool(name="psC", bufs=2, space="PSUM"))   # [128,2,512] scores

    # ---- persistent SBUF tensors ----
    qT = persist.tile([P, MT_Q, QB], BF16)       # q^T [768, QB]
    kT = persist.tile([P, MT_Q, S], BF16)        # k^T [768, S]
    v_sb = persist.tile([P, KT, H, D + 1], BF16)  # v + ones column per head
    o_all = persist.tile([P, H // 2, QB], BF16)   # normalized o^T, head pairs in partition halves
    bq_sb = persist.tile([P, MT_Q], F32)
    bk_sb = persist.tile([P, MT_Q], F32)
    bo_sb = persist.tile([P, E], F32)
    o_raw = persist.tile([D + 1, H, 512], F32)   # unnormalized o^T + denom row per head
    dens = persist.tile([2, H // 2, 512], F32)   # denominators, pair-packed [head-in-pair, hp]
    drec = persist.tile([2, H // 2, 512], F16)   # their reciprocals (fp16 matmul operand)
    sel2 = persist.tile([2, P], F16)             # pair broadcast: [0,0:64]=1, [1,64:128]=1

    # first-needed DMAs first; constants go on the scalar HWDGE queue
    wq_t = wpool.tile([P, EC, E], BF16, tag="w18")
    xq_t = xpool.tile([P, EC, QB], BF16, tag="xs")
    for ec in range(EC):
        nc.sync.dma_start(wq_t[:, ec, :], wq[ec * P:(ec + 1) * P, :])
        nc.sync.dma_start(xq_t[:, ec, :], xq[ec * P:(ec + 1) * P, :])
    nc.scalar.dma_start(bq_sb[:], bq[:])
    nc.scalar.dma_start(bk_sb[:], bk[:])
    nc.scalar.dma_start(bo_sb[:], bo[:])

    # ones columns for denominator (written once; v-proj copies don't touch col D)
    nc.vector.memset(v_sb[:, :, :, D], 1.0)
    # pair-broadcast selector: one K=2 fp16 matmul turns [2,512] reciprocals into
    # a [128,512] per-pair broadcast (head0 rows 0-63, head1 rows 64-127)
    nc.scalar.dma_start(sel2[:], seld[:])

    # ---- q^T projection ----
    for mt in range(MT_Q):
        ps = psA.tile([P, 512], F32, tag="psA")
        for ec in range(EC):
            nc.tensor.matmul(ps[:], wq_t[:, ec, mt * P:(mt + 1) * P], xq_t[:, ec, :],
                             start=(ec == 0), stop=(ec == EC - 1))
        nc.vector.tensor_scalar_add(qT[:, mt, :], ps[:], bq_sb[:, mt:mt + 1])

    # ---- k^T projection ----
    wk_t = wpool.tile([P, EC, E], BF16, tag="w18")
    for ec in range(EC):
        nc.sync.dma_start(wk_t[:, ec, :], wk[ec * P:(ec + 1) * P, :])
    for n4 in range(NC4):
        xk_t = xpool.tile([P, EC, 512], BF16, tag="xs")
        nc.sync.dma_start(xk_t[:], xk[:, n4 * 512:(n4 + 1) * 512].rearrange("(ec p) s -> p ec s", p=P))
        for mt in range(MT_Q):
            ps = psA.tile([P, 512], F32, tag="psA")
            for ec in range(EC):
                nc.tensor.matmul(ps[:], wk_t[:, ec, mt * P:(mt + 1) * P], xk_t[:, ec, :],
                                 start=(ec == 0), stop=(ec == EC - 1))
            nc.vector.tensor_scalar_add(kT[:, mt, n4 * 512:(n4 + 1) * 512], ps[:], bk_sb[:, mt:mt + 1])

    # ---- v projection (direct [keys, d]; no bias — folded into bo host-side) ----
    wv_t = wpool.tile([P, EC, E], BF16, tag="w18")
    for ec in range(EC):
        nc.sync.dma_start(wv_t[:, ec, :], wv[ec * P:(ec + 1) * P, :])
    for kt in range(KT):
        xv_t = xvpool.tile([P, EC, P], BF16, tag="xv")
        nc.sync.dma_start(xv_t[:], xv[:, kt * P:(kt + 1) * P].rearrange("(ec p) s -> p ec s", p=P))
        psv = psB.tile([P, E], F32, tag="psB")
        for ec in range(EC):
            nc.tensor.matmul(psv[:, 0:512], xv_t[:, ec, :], wv_t[:, ec, 0:512],
                             start=(ec == 0), stop=(ec == EC - 1))
            nc.tensor.matmul(psv[:, 512:768], xv_t[:, ec, :], wv_t[:, ec, 512:768],
                             start=(ec == 0), stop=(ec == EC - 1))
        # strided copy into per-head slots (leaves ones column intact)
        nc.vector.tensor_copy(v_sb[:, kt, :, 0:D], psv[:].rearrange("p (h d) -> p h d", d=D))

    # ---- attention: head pairs ----
    # Per key tile: both heads' score matmuls are adjacent K=64 ops on
    # disjoint PE row groups (partitions 0-63 / 64-127) -> run concurrently.
    for hp in range(H // 2):
        o_ps = {}
        for i in range(2):
            o_ps[i] = psA.tile([P, 512], F32, tag="psA", name=f"o_ps{i}")
        for kt in range(KT):
            st = psC.tile([P, 2, 512], F32, tag="psC")
            for i in range(2):
                po = D * i      # partition offset of this head's d-rows
                nc.tensor.matmul(st[:, i, :],
                                 kT[po:po + D, hp, kt * P:(kt + 1) * P],
                                 qT[po:po + D, hp, :],
                                 start=True, stop=True)
            ex = epool.tile([P, 2, 512], BF16, tag="ex")
            nc.scalar.activation(ex[:, :, :], st[:, :, :], mybir.ActivationFunctionType.Exp)
            for i in range(2):
                nc.tensor.matmul(o_ps[i][0:D + 1, :],
                                 v_sb[:, kt, 2 * hp + i, :],
                                 ex[:, i, :],
                                 start=(kt == 0), stop=(kt == KT - 1))
        # stage unnormalized outputs (fast PSUM release), gather denom rows, and
        # compute reciprocals in-stream (DVE/sync only — nothing blocks the PE)
        for i in range(2):
            nc.vector.tensor_copy(o_raw[:, 2 * hp + i, :], o_ps[i][0:D + 1, :])
            nc.sync.dma_start(dens[i:i + 1, hp, :], o_raw[D:D + 1, 2 * hp + i, :])
        with nc.allow_low_precision(reason="fp16 reciprocal feeds fp16 broadcast matmul; den ~1e3, ample range"):
            nc.vector.reciprocal(drec[:, hp, :], dens[:, hp, :])

    # ---- batched softmax normalization (fp16 K=2 matmul broadcasts both
    # reciprocals of a pair into partition halves at once) ----
    for hp in range(H // 2):
        bc_ps = psA.tile([P, 512], F32, tag="psA", name="bc")
        nc.tensor.matmul(bc_ps[:], sel2[:], drec[:, hp, :], start=True, stop=True)
        for i in range(2):
            po = D * i
            nc.vector.tensor_tensor(o_all[po:po + D, hp, :], o_raw[0:D, 2 * hp + i, :],
                                    bc_ps[po:po + D, :], mybir.AluOpType.mult)

    # ---- output projection ----
    wo_t = wpool.tile([P, EC, E], BF16, tag="w18")
    nc.sync.dma_start(wo_t[:], wo[:].rearrange("(ec p) m -> p ec m", p=P))
    ST = QB // P  # 4 s-tiles
    for st4 in range(ST):
        op = psB.tile([P, E], F32, tag="psB")
        for hp in range(H // 2):
            # both heads of the pair contract in one K=128 matmul
            first = (hp == 0)
            last = (hp == H // 2 - 1)
            nc.tensor.matmul(op[:, 0:512],
                             o_all[:, hp, st4 * P:(st4 + 1) * P],
                             wo_t[:, hp, 0:512],
                             start=first, stop=last)
            nc.tensor.matmul(op[:, 512:768],
                             o_all[:, hp, st4 * P:(st4 + 1) * P],
                             wo_t[:, hp, 512:768],
                             start=first, stop=last)
        out_sb = outpool.tile([P, E], F32, tag="outsb")
        nc.vector.tensor_tensor(out_sb[:], op[:], bo_sb[:], mybir.AluOpType.add)
        nc.sync.dma_start(out[st4 * P:(st4 + 1) * P, :], out_sb[:])


_NC_CACHE = None


def _get_nc():
    global _NC_CACHE
    if _NC_CACHE is None:
        _NC_CACHE = build_nc()
    return _NC_CACHE


def make_in_maps(query, key_, value, Wq, bq, Wk, bk, Wv, bv, Wo, bo):
    """Host-side sharding + layout prep. Returns list of 8 input dicts."""
    query = np.asarray(query, dtype=np.float32)
    key_ = np.asarray(key_, dtype=np.float32)
    value = np.asarray(value, dtype=np.float32)
    scale = 1.0 / np.sqrt(np.float32(D))

    import ml_dtypes
    BF = ml_dtypes.bfloat16
    wq_f = (np.ascontiguousarray(np.transpose(np.asarray(Wq, np.float32), (1, 0, 2)).reshape(E, E)) * scale).astype(BF)
    wk_f = np.ascontiguousarray(np.transpose(np.asarray(Wk, np.float32), (1, 0, 2)).reshape(E, E)).astype(BF)
    wv_f = np.ascontiguousarray(np.transpose(np.asarray(Wv, np.float32), (1, 0, 2)).reshape(E, E)).astype(BF)
    wo_f = np.ascontiguousarray(np.asarray(Wo, np.float32)).astype(BF)

    bq_f = (np.asarray(bq, np.float32).reshape(E) * scale).reshape(MT_Q, P).T.copy()
    bk_f = np.asarray(bk, np.float32).reshape(E).reshape(MT_Q, P).T.copy()
    bv_f = np.asarray(bv, np.float32).reshape(E)
    wo_f32 = wo_f.astype(np.float32)
    bo_eff = np.tile((bv_f @ wo_f32 + np.asarray(bo, np.float32)).reshape(1, E), (P, 1)).copy()

    xk_t = [np.ascontiguousarray(key_[b].T).astype(BF) for b in range(B)]
    xv_t = [np.ascontiguousarray(value[b].T).astype(BF) for b in range(B)]

    sel_np = np.zeros((2, P), dtype=np.float16)
    sel_np[0, 0:D] = 1.0
    sel_np[1, D:2 * D] = 1.0

    in_maps = []
    for core in range(NCORES):
        b = core // (NCORES // B)
        qc = core % (NCORES // B)
        xq_t = np.ascontiguousarray(query[b, qc * QB:(qc + 1) * QB, :].T).astype(BF)
        in_maps.append({
            "xq": xq_t, "xk": xk_t[b], "xv": xv_t[b],
            "wq": wq_f, "wk": wk_f, "wv": wv_f, "wo": wo_f,
            "bq": bq_f, "bk": bk_f, "bo": bo_eff, "seld": sel_np,
        })
    return in_maps


def assemble(results):
    outp = np.empty((B, S, E), dtype=np.float32)
    for core in range(NCORES):
        b = core // (NCORES // B)
        qc = core % (NCORES // B)
        outp[b, qc * QB:(qc + 1) * QB, :] = results[core]["out"]
    return outp


def kernel(query, key_, value, Wq, bq, Wk, bk, Wv, bv, Wo, bo):
    nc = _get_nc()
    in_maps = make_in_maps(query, key_, value, Wq, bq, Wk, bk, Wv, bv, Wo, bo)
    res = run_bass_kernel_spmd(nc, in_maps, core_ids=list(range(NCORES)))
    return assemble(res.results)

